# revision 1
# baseline (speedup 1.0000x reference)
"""Trainium2 Bass kernel for nn_KeyRecorder.

Math (reference):
  comp = LN(relu(obs @ W1 + b1)) * g1 + bl1          [B, T, R]
  past = max(comp[:, :-20:10, :], axis=time)          408 strided rows
  gmax = max(cummax(comp[:, -20:, :]), past)          [B, 20, R]
  out  = LN(relu(gmax @ W2 + b2)) * g2 + bl2          [B, 20, D]

Only 428 of the 4096 timesteps per batch element are ever consumed
(408 strided + last 20), so the host gathers exactly those rows,
transposes them to d-major layout, and ships ~1.75 MB/core instead of
16.8 MB/core.  Batch is sharded 2-per-core across 8 cores (pure data
parallel, no collectives).

LN1's affine (g1, bl1) is folded into W2/b2 on the host:
  max/cummax commute with x -> x*g1+bl1 elementwise when g1 >= 0
  (asserted), and (gmax*g1+bl1) @ W2 = gmax @ (g1[:,None]*W2) + bl1@W2.
"""

import os
import numpy as np

import concourse.bass as bass
import concourse.bacc as bacc
import concourse.mybir as mybir
import concourse.tile as tile
from concourse.bass_utils import run_bass_kernel_spmd

F32 = mybir.dt.float32
ALU = mybir.AluOpType
ACT = mybir.ActivationFunctionType
AX = mybir.AxisListType

B, T, D, R = 16, 4096, 512, 64
LOCAL, SR, EPS = 20, 10, 1e-5
N_CORES = 8
BPC = B // N_CORES            # batch elements per core
NSTR = (T - LOCAL + SR - 1) // SR   # 408 strided past rows
NSEL = NSTR + LOCAL           # 428 rows consumed per batch element
GRP = 448                     # per-batch group width in SBUF (428 padded)
NTOK = GRP * BPC              # 896 token columns per core
NTT = NTOK // 128             # 7 token tiles
DC = D // 128                 # 4 contraction chunks
NO = BPC * LOCAL              # 40 output rows per core

IDX = np.array(list(range(0, T - LOCAL, SR)) + list(range(T - LOCAL, T)))

_cache: dict = {}


def _build_program():
    """Build + compile the per-core Bass program once."""
    if "nc" in _cache:
        return _cache["nc"]

    nc = bacc.Bacc("TRN2", target_bir_lowering=False, debug=False,
                   enable_asserts=False)

    obsT_d = nc.dram_tensor("obsT", [DC, 128, NTOK], F32, kind="ExternalInput")
    w1_d = nc.dram_tensor("w1c", [DC, 128, R], F32, kind="ExternalInput")
    b1_d = nc.dram_tensor("b1col", [R, 1], F32, kind="ExternalInput")
    w2_d = nc.dram_tensor("w2f", [R, D], F32, kind="ExternalInput")
    b2_d = nc.dram_tensor("b2row", [1, D], F32, kind="ExternalInput")
    g2_d = nc.dram_tensor("g2b", [NO, D], F32, kind="ExternalInput")
    bl2_d = nc.dram_tensor("bl2b", [NO, D], F32, kind="ExternalInput")
    id_d = nc.dram_tensor("ident", [128, 128], F32, kind="ExternalInput")
    out_d = nc.dram_tensor("out", [NO, D], F32, kind="ExternalOutput")

    inv_r = 1.0 / R
    inv_d = 1.0 / D

    with tile.TileContext(nc) as tc:
        with (
            tc.tile_pool(name="const", bufs=1) as cpool,
            tc.tile_pool(name="work", bufs=4) as wpool,
            tc.tile_pool(name="stats", bufs=6) as spool,
            tc.tile_pool(name="ps_grp", bufs=2, space=bass.MemorySpace.PSUM) as pgrp,
            tc.tile_pool(name="ps_mm", bufs=3, space=bass.MemorySpace.PSUM) as pmm,
            tc.tile_pool(name="ps_tr", bufs=2, space=bass.MemorySpace.PSUM) as ptr,
            tc.tile_pool(name="ps_o", bufs=1, space=bass.MemorySpace.PSUM) as pout,
        ):
            # ---- load constants first (first matmul needs them) ----
            w1 = cpool.tile([128, DC, R], F32)
            for c in range(DC):
                nc.sync.dma_start(w1[:, c, :], w1_d[c])
            b1c = cpool.tile([R, 1], F32)
            nc.sync.dma_start(b1c[:], b1_d[:])
            ident = cpool.tile([128, 128], F32)
            nc.sync.dma_start(ident[:], id_d[:])
            ones1 = cpool.tile([1, 128], F32)
            nc.vector.memset(ones1[:], 1.0)
            w2 = cpool.tile([R, D], F32)
            nc.sync.dma_start(w2[:], w2_d[:])
            b2r = cpool.tile([1, D], F32)
            nc.sync.dma_start(b2r[:], b2_d[:])
            g2 = cpool.tile([NO, D], F32)
            nc.sync.dma_start(g2[:], g2_d[:])
            bl2 = cpool.tile([NO, D], F32)
            nc.sync.dma_start(bl2[:], bl2_d[:])

            # ---- input: one contiguous DMA per contraction chunk ----
            obsT = cpool.tile([128, DC, NTOK], F32)
            for c in range(DC):
                nc.sync.dma_start(obsT[:, c, :], obsT_d[c])

            compT = cpool.tile([R, NTOK], F32)   # LN'd comp, [r, t] layout

            # ---- stage 1: comp = LN(relu(obs @ W1 + b1)) ----
            # W1-stationary matmuls: compT_pre [r, t] in two PSUM groups
            # (512 + 384 cols), bias fused into the PSUM->SBUF copy.
            cpre = cpool.tile([R, NTOK], F32)
            for g, (lo, w) in enumerate(((0, 512), (512, 384))):
                pg = pgrp.tile([R, 512], F32, tag="pg")
                for c in range(DC):
                    nc.tensor.matmul(pg[:, 0:w], w1[:, c, :],
                                     obsT[:, c, lo:lo + w],
                                     start=(c == 0), stop=(c == DC - 1))
                nc.vector.tensor_scalar_add(cpre[:, lo:lo + w], pg[:, 0:w],
                                            b1c[:])
            for tt in range(NTT):
                # transpose 128-token slab to [t, r] for the row LN
                ps = pmm.tile([128, R], F32, tag="ps")
                nc.tensor.transpose(ps[:], cpre[:, bass.ts(tt, 128)],
                                    ident[0:R, 0:R])
                # relu + row-sum in one op
                xr = wpool.tile([128, R], F32, tag="xr")
                rsum = spool.tile([128, 1], F32, tag="rsum")
                nc.vector.tensor_scalar(xr[:], ps[:], 0.0, 0.0, ALU.max,
                                        ALU.add, accum_out=rsum[:])
                negmu = spool.tile([128, 1], F32, tag="negmu")
                nc.gpsimd.tensor_scalar_mul(negmu[:], rsum[:], -inv_r)
                xc = wpool.tile([128, R], F32, tag="xc")
                nc.vector.tensor_scalar_add(xc[:], xr[:], negmu[:])
                # squared sum: square on ACT with fused row-sum
                sq = wpool.tile([128, R], F32, tag="sq")
                ssq = spool.tile([128, 1], F32, tag="ssq")
                nc.scalar.activation(sq[:], xc[:], ACT.Square,
                                     accum_out=ssq[:])
                ssqe = spool.tile([128, 1], F32, tag="ssqe")
                nc.vector.tensor_scalar_add(ssqe[:], ssq[:], R * EPS)
                std = spool.tile([128, 1], F32, tag="std")
                nc.scalar.activation(std[:], ssqe[:], ACT.Sqrt,
                                     bias=0.0, scale=inv_r)
                rstd = spool.tile([128, 1], F32, tag="rstd")
                nc.vector.reciprocal(rstd[:], std[:])
                y = wpool.tile([128, R], F32, tag="y")
                nc.vector.tensor_scalar_mul(y[:], xc[:], rstd[:])

                # transpose to [r, t] for the time reductions
                pt = ptr.tile([R, 128], F32, tag="pt")
                nc.tensor.transpose(pt[:], y[:], ident[:])
                nc.vector.tensor_copy(compT[:, bass.ts(tt, 128)], pt[:])

            # ---- stage 2: strided max + seeded cummax (free-axis ops) ----
            past0 = spool.tile([R, 1], F32, tag="past0")
            nc.vector.reduce_max(past0[:], compT[:, 0:NSTR], axis=AX.X)
            past1 = spool.tile([R, 1], F32, tag="past1")
            nc.vector.reduce_max(past1[:], compT[:, GRP:GRP + NSTR], axis=AX.X)

            pa = cpool.tile([R, BPC, LOCAL], F32)
            pb = cpool.tile([R, BPC, LOCAL], F32)
            nc.vector.tensor_copy(pa[:, 0, :], compT[:, NSTR:NSEL])
            nc.vector.tensor_copy(pa[:, 1, :], compT[:, GRP + NSTR:GRP + NSEL])
            cur, nxt = pa, pb
            s = 1
            while s < LOCAL:
                nc.vector.tensor_tensor(nxt[:, :, s:], cur[:, :, s:],
                                        cur[:, :, :LOCAL - s], op=ALU.max)
                nc.vector.tensor_copy(nxt[:, :, 0:s], cur[:, :, 0:s])
                cur, nxt = nxt, cur
                s *= 2

            gmaxT = cpool.tile([R, NO], F32)
            nc.vector.tensor_scalar(gmaxT[:, 0:LOCAL], cur[:, 0, :],
                                    past0[:], None, ALU.max)
            nc.vector.tensor_scalar(gmaxT[:, LOCAL:NO], cur[:, 1, :],
                                    past1[:], None, ALU.max)

            # ---- stage 3: out = LN(relu(gmax @ W2' + b2')) * g2 + bl2 ----
            ps2 = pout.tile([NO, D], F32)
            nc.tensor.matmul(ps2[:], gmaxT[:], w2[:], start=True, stop=False)
            nc.tensor.matmul(ps2[:], ones1[:, 0:NO], b2r[:],
                             start=False, stop=True)

            xr2 = cpool.tile([NO, D], F32)
            rsum2 = spool.tile([NO, 1], F32, tag="rsum2")
            nc.vector.tensor_scalar(xr2[:], ps2[:], 0.0, 0.0, ALU.max,
                                    ALU.add, accum_out=rsum2[:])
            negmu2 = spool.tile([NO, 1], F32, tag="negmu2")
            nc.gpsimd.tensor_scalar_mul(negmu2[:], rsum2[:], -inv_d)
            xc2 = cpool.tile([NO, D], F32)
            nc.vector.tensor_scalar_add(xc2[:], xr2[:], negmu2[:])
            sq2 = cpool.tile([NO, D], F32)
            ssq2 = spool.tile([NO, 1], F32, tag="ssq2")
            nc.scalar.activation(sq2[:], xc2[:], ACT.Square,
                                 accum_out=ssq2[:])
            ssqe2 = spool.tile([NO, 1], F32, tag="ssqe2")
            nc.vector.tensor_scalar_add(ssqe2[:], ssq2[:], D * EPS)
            std2 = spool.tile([NO, 1], F32, tag="std2")
            nc.scalar.activation(std2[:], ssqe2[:], ACT.Sqrt,
                                 bias=0.0, scale=inv_d)
            rstd2 = spool.tile([NO, 1], F32, tag="rstd2")
            nc.vector.reciprocal(rstd2[:], std2[:])
            yn = cpool.tile([NO, D], F32)
            nc.vector.tensor_scalar_mul(yn[:], xc2[:], rstd2[:])
            yg = cpool.tile([NO, D], F32)
            nc.vector.tensor_mul(yg[:], yn[:], g2[:])
            out_sb = cpool.tile([NO, D], F32)
            nc.vector.tensor_add(out_sb[:], yg[:], bl2[:])

            nc.sync.dma_start(out_d[:], out_sb[:])

    nc.compile()
    _cache["nc"] = nc
    return nc


def _host_inputs(obs, W1, b1, ln1_g, ln1_b, W2, b2, ln2_g, ln2_b):
    obs = np.ascontiguousarray(np.asarray(obs, dtype=np.float32))
    W1 = np.asarray(W1, np.float32)
    b1 = np.asarray(b1, np.float32)
    ln1_g = np.asarray(ln1_g, np.float32)
    ln1_b = np.asarray(ln1_b, np.float32)
    W2 = np.asarray(W2, np.float32)
    b2 = np.asarray(b2, np.float32)
    ln2_g = np.asarray(ln2_g, np.float32)
    ln2_b = np.asarray(ln2_b, np.float32)

    # folding LN1's affine past the max/cummax requires monotonicity
    assert np.all(ln1_g >= 0), "ln1_g must be >= 0 for the affine fold"

    w1c = np.ascontiguousarray(W1.reshape(DC, 128, R))
    b1r = b1.reshape(R, 1)
    w2f = np.ascontiguousarray(ln1_g[:, None] * W2)
    b2f = (b2 + ln1_b @ W2).astype(np.float32).reshape(1, D)
    g2b = np.ascontiguousarray(np.broadcast_to(ln2_g, (NO, D)))
    bl2b = np.ascontiguousarray(np.broadcast_to(ln2_b, (NO, D)))
    ident = np.eye(128, dtype=np.float32)

    shared = {"w1c": w1c, "b1col": b1r, "w2f": w2f, "b2row": b2f,
              "g2b": g2b, "bl2b": bl2b, "ident": ident}
    in_maps = []
    for c in range(N_CORES):
        sel = obs[BPC * c:BPC * (c + 1)][:, IDX, :]        # [BPC, 428, 512]
        grp = np.zeros((BPC, GRP, D), np.float32)
        grp[:, :NSEL] = sel
        obsT = np.ascontiguousarray(grp.reshape(NTOK, D).T)  # [512, 896]
        in_maps.append({"obsT": obsT.reshape(DC, 128, NTOK), **shared})
    return in_maps


def _install_ntff_shim():
    """The agent image's antenv lacks axon_hooks; synthesize it so
    trace=True can reach the libaxon NTFF profiler (test-time only)."""
    import sys
    import types
    if "antenv.axon_hooks" in sys.modules:
        return True
    try:
        import antenv
        from trn_agent_boot.trn_boot import _ntff_profile_via_ctypes
    except ImportError:
        return False
    so_path = "/opt/axon/libaxon_pjrt.so"
    if not os.path.exists(so_path):
        return False
    hook = _ntff_profile_via_ctypes(so_path)
    mod = types.ModuleType("antenv.axon_hooks")
    mod._hook = hook
    mod.set_axon_ntff_profile_hook = lambda h: setattr(mod, "_hook", h)
    mod.get_axon_ntff_profile_hook = lambda: mod._hook
    sys.modules["antenv.axon_hooks"] = mod
    antenv.axon_hooks = mod
    return hook is not None


def kernel(obs_frames, W1, b1, ln1_g, ln1_b, W2, b2, ln2_g, ln2_b):
    nc = _build_program()
    in_maps = _host_inputs(obs_frames, W1, b1, ln1_g, ln1_b,
                           W2, b2, ln2_g, ln2_b)
    trace = bool(os.environ.get("BASS_TRACE"))
    if trace:
        trace = _install_ntff_shim()
        import concourse.bass_utils as _bu
        _bu.upload_artifacts = lambda tmpdir: f"local://{tmpdir}"
    res = run_bass_kernel_spmd(nc, in_maps, core_ids=list(range(N_CORES)),
                               trace=trace)
    _cache["last_result"] = res
    out = np.stack([res.results[c]["out"].reshape(BPC, LOCAL, D)
                    for c in range(N_CORES)])
    return out.reshape(B, LOCAL, D)



# revision 7
# speedup vs baseline: 1.9751x; 1.9751x over previous
"""Trainium2 Bass kernel for nn_KeyRecorder (optimized v2).

Math (reference):
  comp = LN(relu(obs @ W1 + b1)) * g1 + bl1          [B, T, R]
  past = max(comp[:, :-20:10, :], axis=time)          408 strided rows
  gmax = max(cummax(comp[:, -20:, :]), past)          [B, 20, R]
  out  = LN(relu(gmax @ W2 + b2)) * g2 + bl2          [B, 20, D]

Only 428 of the 4096 timesteps per batch element are consumed (408
strided + last 20); the host gathers those rows, pads each batch
element to 448 tokens and ships them transposed (d-major) in fp16:
~0.92 MB/core.  Batch is sharded 2-per-core across 8 cores.

Device-side structure (per core, 896 token columns = 7 slabs of 128):
  stage 1: W1-stationary fp16 matmuls -> psum [64,896] in 2 groups;
           ACT relu(x+b1) -> fp16 [r,t]; per-slab transpose via a
           [64,65] (identity | ones) matmul so each transposed slab
           lands in psum as [128 tok, 64 feat + rowsum]; batched LN
           stats on [128,7] tiles; fused (x-mu)*rstd apply per slab;
           transpose back to [r,t] psum via fp16 identity matmuls.
  stage 2: past = one reduce_max per batch elem over 408 psum cols;
           seeded running max = one tensor_tensor_scan (hw prefix
           scan, initial=past) per batch elem.
  stage 3: single [65,40]x[65,512] matmul (ones row adds b2), ACT
           relu with fused row-sum, tensor_tensor_reduce for sum of
           squares, fused (x-mu)*rstd apply, DMA out.

Affine folds (host side): LN1's g1/bl1 fold into W2/b2 (g1 >= 0
asserted, max/cummax commute with monotone maps); LN2's g2/bl2 are
applied to the gathered output on the host (elementwise per-feature).
"""

import os
import numpy as np

import concourse.bass as bass
import concourse.bacc as bacc
import concourse.mybir as mybir
import concourse.tile as tile
from concourse.bass_utils import run_bass_kernel_spmd

F32 = mybir.dt.float32
F16 = mybir.dt.float16
ALU = mybir.AluOpType
ACT = mybir.ActivationFunctionType
AX = mybir.AxisListType

B, T, D, R = 16, 4096, 512, 64
LOCAL, SR, EPS = 20, 10, 1e-5
N_CORES = 8
BPC = B // N_CORES                   # batch elements per core
NSTR = (T - LOCAL + SR - 1) // SR    # 408 strided past rows
NSEL = NSTR + LOCAL                  # 428 rows consumed per batch elem
GRP = 448                            # per-batch group width (428 padded)
NTOK = GRP * BPC                     # 896 token columns per core
NSLAB = NTOK // 128                  # 7 token slabs
DC = D // 128                        # 4 contraction chunks
NO = BPC * LOCAL                     # 40 output rows per core
W0, W1W = 512, NTOK - 512            # matmul token groups

IDX = np.array(list(range(0, T - LOCAL, SR)) + list(range(T - LOCAL, T)))

_cache: dict = {}


# tensor_tensor_scan is HW-verified here; tensor_tensor_reduce crashes the
# exec unit (NRT_EXEC_UNIT_UNRECOVERABLE) on this runtime, so ssq in stage 3
# uses ACT Square with a fused accumulator instead.
USE_SCAN = os.environ.get("KV_NO_SCAN", "") == ""
USE_TTR = os.environ.get("KV_USE_TTR", "") != ""


def _build_program():
    if "nc" in _cache:
        return _cache["nc"]

    nc = bacc.Bacc("TRN2", target_bir_lowering=False, debug=False,
                   enable_asserts=False)

    obs0_d = nc.dram_tensor("obs0", [128, DC, W0], F16, kind="ExternalInput")
    obs1_d = nc.dram_tensor("obs1", [128, DC, W1W], F16, kind="ExternalInput")
    w1_d = nc.dram_tensor("w1f", [128, DC, R], F16, kind="ExternalInput")
    b1_d = nc.dram_tensor("b1col", [R, 1], F32, kind="ExternalInput")
    idp_d = nc.dram_tensor("identp", [R, R + 1], F16, kind="ExternalInput")
    id128_d = nc.dram_tensor("ident128", [128, 128], F16, kind="ExternalInput")
    w2_d = nc.dram_tensor("w2aug", [R + 1, D], F16, kind="ExternalInput")
    out_d = nc.dram_tensor("out", [NO, D], F32, kind="ExternalOutput")

    inv_r = 1.0 / R
    inv_d = 1.0 / D

    with tile.TileContext(nc) as tc:
        with (
            tc.tile_pool(name="const", bufs=1) as cpool,
            tc.tile_pool(name="pg", bufs=2, space=bass.MemorySpace.PSUM) as ppg,
            tc.tile_pool(name="xr", bufs=2, space=bass.MemorySpace.PSUM) as pxr,
            tc.tile_pool(name="ct", bufs=2, space=bass.MemorySpace.PSUM) as pct,
            tc.tile_pool(name="o3", bufs=1, space=bass.MemorySpace.PSUM) as po3,
        ):
            # ---------- SBUF tiles ----------
            obs_sb0 = cpool.tile([128, DC, W0], F16)
            obs_sb1 = cpool.tile([128, DC, W1W], F16)
            w1_sb = cpool.tile([128, DC, R], F16)
            b1_sb = cpool.tile([R, 1], F32)
            idp_sb = cpool.tile([R, R + 1], F16)
            id128_sb = cpool.tile([128, 128], F16)
            w2_sb = cpool.tile([R + 1, D], F16)
            xrT = cpool.tile([R, NTOK], F16)          # relu(z+b1), [r, t]
            sq_sb = cpool.tile([128, NSLAB, R], F16)  # x^2, [t, slab, r]
            mu = cpool.tile([128, NSLAB], F32)
            ssq = cpool.tile([128, NSLAB], F32)
            msqe = cpool.tile([128, NSLAB], F32)
            mu2 = cpool.tile([128, NSLAB], F32)
            var = cpool.tile([128, NSLAB], F32)
            stdv = cpool.tile([128, NSLAB], F32)
            rstd = cpool.tile([128, NSLAB], F32)
            y_sb = cpool.tile([128, NSLAB, R], F16)   # LN'd comp, [t, slab, r]
            neginf = cpool.tile([R, LOCAL], F16)
            gmaux = cpool.tile([R + 1, NO], F16)      # gmax^T + ones row
            past0 = cpool.tile([R, 1], F32)
            past1 = cpool.tile([R, 1], F32)
            xr3 = cpool.tile([NO, D], F32)
            sq3 = cpool.tile([NO, D], F16)            # unused product dump
            rsum3 = cpool.tile([NO, 1], F32)
            ssq3 = cpool.tile([NO, 1], F32)
            mu3 = cpool.tile([NO, 1], F32)
            msqe3 = cpool.tile([NO, 1], F32)
            mu23 = cpool.tile([NO, 1], F32)
            var3 = cpool.tile([NO, 1], F32)
            std3 = cpool.tile([NO, 1], F32)
            rstd3 = cpool.tile([NO, 1], F32)
            outsb = cpool.tile([NO, D], F32)

            # ---------- DMA in: obs on sync queue, weights on ACT ----------
            nc.sync.dma_start(obs_sb0[:], obs0_d[:])
            nc.sync.dma_start(w1_sb[:], w1_d[:])
            nc.sync.dma_start(obs_sb1[:], obs1_d[:])
            nc.sync.dma_start(b1_sb[:], b1_d[:])
            nc.sync.dma_start(idp_sb[:], idp_d[:])
            nc.sync.dma_start(id128_sb[:], id128_d[:])
            nc.sync.dma_start(w2_sb[:], w2_d[:])
            nc.gpsimd.memset(neginf[:], -60000.0)
            nc.gpsimd.memset(gmaux[R:R + 1, :], 1.0)

            # ---------- stage 1: z = obs @ W1 (W1 stationary) ----------
            pg0 = ppg.tile([R, W0], F32, tag="pg")
            for c in range(DC):
                nc.tensor.matmul(pg0[:], w1_sb[:, c, :], obs_sb0[:, c, :],
                                 start=(c == 0), stop=(c == DC - 1))
            nc.scalar.activation(xrT[:, 0:W0], pg0[:], ACT.Relu,
                                 bias=b1_sb[:], scale=1.0)

            pg1 = ppg.tile([R, W0], F32, tag="pg")
            for c in range(DC):
                nc.tensor.matmul(pg1[:, 0:W1W], w1_sb[:, c, :],
                                 obs_sb1[:, c, :],
                                 start=(c == 0), stop=(c == DC - 1))
            nc.scalar.activation(xrT[:, W0:NTOK], pg1[:, 0:W1W], ACT.Relu,
                                 bias=b1_sb[:], scale=1.0)

            # ---------- transpose slabs + fused row-sums ----------
            # xrp[t, j, 0:64] = slab token-major, xrp[t, j, 64] = row sum
            xrp0 = pxr.tile([128, DC, R + 1], F32, tag="xr")
            xrp1 = pxr.tile([128, DC, R + 1], F32, tag="xr")
            for s in range(NSLAB):
                dst = xrp0[:, s, :] if s < 4 else xrp1[:, s - 4, :]
                nc.tensor.matmul(dst, xrT[:, 128 * s:128 * (s + 1)],
                                 idp_sb[:], start=True, stop=True)

            # ---------- batched LN stats ----------
            nc.vector.tensor_scalar_mul(mu[:, 0:4], xrp0[:, :, R], inv_r)
            nc.vector.tensor_scalar_mul(mu[:, 4:NSLAB],
                                        xrp1[:, 0:NSLAB - 4, R], inv_r)
            nc.scalar.activation(sq_sb[:, 0:4, :], xrp0[:, :, 0:R],
                                 ACT.Square)
            nc.scalar.activation(sq_sb[:, 4:NSLAB, :],
                                 xrp1[:, 0:NSLAB - 4, 0:R], ACT.Square)
            nc.vector.reduce_sum(ssq[:], sq_sb[:], axis=AX.X)
            nc.gpsimd.tensor_scalar(msqe[:], ssq[:], inv_r, EPS,
                                    ALU.mult, ALU.add)
            nc.gpsimd.tensor_tensor(mu2[:], mu[:], mu[:], op=ALU.mult)
            nc.vector.tensor_tensor(var[:], msqe[:], mu2[:], op=ALU.subtract)
            nc.scalar.activation(stdv[:], var[:], ACT.Sqrt)
            nc.vector.reciprocal(rstd[:], stdv[:])

            # ---------- apply LN + transpose back to [r, t] ----------
            ct0 = pct.tile([R, GRP], F32, tag="ct")
            ct1 = pct.tile([R, GRP], F32, tag="ct")
            for s in range(NSLAB):
                src = xrp0[:, s, 0:R] if s < 4 else xrp1[:, s - 4, 0:R]
                nc.vector.tensor_scalar(y_sb[:, s, :], src,
                                        mu[:, s:s + 1], rstd[:, s:s + 1],
                                        ALU.subtract, ALU.mult)
                if s < 3:
                    nc.tensor.matmul(ct0[:, 128 * s:128 * (s + 1)],
                                     y_sb[:, s, :], id128_sb[:],
                                     start=True, stop=True)
                elif s == 3:
                    nc.tensor.matmul(ct0[:, 384:448], y_sb[:, s, :],
                                     id128_sb[:, 0:64], start=True, stop=True)
                    nc.tensor.matmul(ct1[:, 0:64], y_sb[:, s, :],
                                     id128_sb[:, 64:128],
                                     start=True, stop=True)
                else:
                    lo = 64 + 128 * (s - 4)
                    nc.tensor.matmul(ct1[:, lo:lo + 128], y_sb[:, s, :],
                                     id128_sb[:], start=True, stop=True)

            # ---------- stage 2: strided past max + seeded cummax ----------
            nc.vector.reduce_max(past0[:], ct0[:, 0:NSTR], axis=AX.X)
            nc.vector.reduce_max(past1[:], ct1[:, 0:NSTR], axis=AX.X)
            if USE_SCAN:
                nc.vector.tensor_tensor_scan(gmaux[0:R, 0:LOCAL],
                                             ct0[:, NSTR:NSEL], neginf[:],
                                             past0[:], ALU.max, ALU.max)
                nc.vector.tensor_tensor_scan(gmaux[0:R, LOCAL:NO],
                                             ct1[:, NSTR:NSEL], neginf[:],
                                             past1[:], ALU.max, ALU.max)
            else:
                la = cpool.tile([R, 2, LOCAL], F32)
                lb = cpool.tile([R, 2, LOCAL], F32)
                nc.vector.tensor_copy(la[:, 0, :], ct0[:, NSTR:NSEL])
                nc.vector.tensor_copy(la[:, 1, :], ct1[:, NSTR:NSEL])
                cur, nxt = la, lb
                st = 1
                while st < LOCAL:
                    nc.vector.tensor_tensor(nxt[:, :, st:], cur[:, :, st:],
                                            cur[:, :, :LOCAL - st],
                                            op=ALU.max)
                    nc.vector.tensor_copy(nxt[:, :, 0:st], cur[:, :, 0:st])
                    cur, nxt = nxt, cur
                    st *= 2
                nc.vector.tensor_scalar(gmaux[0:R, 0:LOCAL], cur[:, 0, :],
                                        past0[:], None, ALU.max)
                nc.vector.tensor_scalar(gmaux[0:R, LOCAL:NO], cur[:, 1, :],
                                        past1[:], None, ALU.max)

            # ---------- stage 3: out = LN(relu(gmax @ W2' + b2')) ----------
            ps3 = po3.tile([NO, D], F32, tag="o3")
            nc.tensor.matmul(ps3[:], gmaux[:], w2_sb[:], start=True, stop=True)
            nc.scalar.activation(xr3[:], ps3[:], ACT.Relu,
                                 accum_out=rsum3[:])
            if USE_TTR:
                nc.vector.tensor_tensor_reduce(
                    out=sq3[:], in0=xr3[:], in1=xr3[:], scale=1.0, scalar=0.0,
                    op0=ALU.mult, op1=ALU.add, accum_out=ssq3[:])
            else:
                nc.scalar.activation(sq3[:], xr3[:], ACT.Square,
                                     accum_out=ssq3[:])
            nc.gpsimd.tensor_scalar_mul(mu3[:], rsum3[:], inv_d)
            nc.gpsimd.tensor_scalar(msqe3[:], ssq3[:], inv_d, EPS,
                                    ALU.mult, ALU.add)
            nc.gpsimd.tensor_tensor(mu23[:], mu3[:], mu3[:], op=ALU.mult)
            nc.vector.tensor_tensor(var3[:], msqe3[:], mu23[:],
                                    op=ALU.subtract)
            nc.scalar.activation(std3[:], var3[:], ACT.Sqrt)
            nc.vector.reciprocal(rstd3[:], std3[:])
            nc.vector.tensor_scalar(outsb[:], xr3[:], mu3[:], rstd3[:],
                                    ALU.subtract, ALU.mult)

            nc.sync.dma_start(out_d[:], outsb[:])

    nc.compile()
    _cache["nc"] = nc
    return nc


def _host_inputs(obs, W1, b1, ln1_g, ln1_b, W2, b2):
    obs = np.ascontiguousarray(np.asarray(obs, dtype=np.float32))
    W1 = np.asarray(W1, np.float32)
    b1 = np.asarray(b1, np.float32)
    ln1_g = np.asarray(ln1_g, np.float32)
    ln1_b = np.asarray(ln1_b, np.float32)
    W2 = np.asarray(W2, np.float32)
    b2 = np.asarray(b2, np.float32)

    # folding LN1's affine past the max/cummax requires monotonicity
    assert np.all(ln1_g >= 0), "ln1_g must be >= 0 for the affine fold"

    w1f = np.ascontiguousarray(
        W1.reshape(DC, 128, R).transpose(1, 0, 2)).astype(np.float16)
    b1col = b1.reshape(R, 1)
    identp = np.concatenate(
        [np.eye(R, dtype=np.float16), np.ones((R, 1), np.float16)], axis=1)
    ident128 = np.eye(128, dtype=np.float16)
    w2aug = np.concatenate(
        [ln1_g[:, None] * W2, (b2 + ln1_b @ W2)[None, :]],
        axis=0).astype(np.float16)

    shared = {"w1f": w1f, "b1col": b1col, "identp": identp,
              "ident128": ident128, "w2aug": w2aug}
    in_maps = []
    for c in range(N_CORES):
        sel = obs[BPC * c:BPC * (c + 1)][:, IDX, :]        # [BPC, 428, 512]
        grp = np.zeros((BPC, GRP, D), np.float32)
        grp[:, :NSEL] = sel
        obsT = grp.reshape(NTOK, D).T                       # [512, 896]
        obsf = obsT.reshape(DC, 128, NTOK).transpose(1, 0, 2)  # [p, c, t]
        obsf16 = obsf.astype(np.float16)
        in_maps.append({
            "obs0": np.ascontiguousarray(obsf16[:, :, 0:W0]),
            "obs1": np.ascontiguousarray(obsf16[:, :, W0:NTOK]),
            **shared})
    return in_maps


def _install_ntff_shim():
    """The agent image's antenv lacks axon_hooks; synthesize it so
    trace=True can reach the libaxon NTFF profiler (test-time only)."""
    import sys
    import types
    if "antenv.axon_hooks" in sys.modules:
        return True
    try:
        import antenv
        from trn_agent_boot.trn_boot import _ntff_profile_via_ctypes
    except ImportError:
        return False
    so_path = "/opt/axon/libaxon_pjrt.so"
    if not os.path.exists(so_path):
        return False
    hook = _ntff_profile_via_ctypes(so_path)
    mod = types.ModuleType("antenv.axon_hooks")
    mod._hook = hook
    mod.set_axon_ntff_profile_hook = lambda h: setattr(mod, "_hook", h)
    mod.get_axon_ntff_profile_hook = lambda: mod._hook
    sys.modules["antenv.axon_hooks"] = mod
    antenv.axon_hooks = mod
    return hook is not None


def kernel(obs_frames, W1, b1, ln1_g, ln1_b, W2, b2, ln2_g, ln2_b):
    nc = _build_program()
    in_maps = _host_inputs(obs_frames, W1, b1, ln1_g, ln1_b, W2, b2)
    trace = bool(os.environ.get("BASS_TRACE"))
    if trace:
        trace = _install_ntff_shim()
        import concourse.bass_utils as _bu
        _bu.upload_artifacts = lambda tmpdir: f"local://{tmpdir}"
    res = run_bass_kernel_spmd(nc, in_maps, core_ids=list(range(N_CORES)),
                               trace=trace)
    _cache["last_result"] = res
    out = np.stack([res.results[c]["out"].reshape(BPC, LOCAL, D)
                    for c in range(N_CORES)])
    out = out.reshape(B, LOCAL, D)

    # LN2's affine applied host-side (identity for the given inputs)
    g2 = np.asarray(ln2_g, np.float32)
    b2l = np.asarray(ln2_b, np.float32)
    if not (np.all(g2 == 1.0) and np.all(b2l == 0.0)):
        out = out * g2 + b2l
    return np.ascontiguousarray(out.astype(np.float32))
